# revision 21
# baseline (speedup 1.0000x reference)
"""Trainium2 Bass kernel for spatial self-attention block.

Reference computation (per batch element):
    xn = GroupNorm32(x); tokens = xn reshaped [n=h*w, c]
    qkv = tokens @ w_qkv.T + b_qkv ; scores = q @ k.T * c**-0.5
    out = softmax(scores) @ v ; out = out @ w_out.T + b_out ; out + x

Sharding: 8 cores, core i handles batch i//2, query-rows half i%2 of the
4096 tokens (2048 queries per core). The host rotates the token axis per
core so every core's queries are tokens [0, 2048) of ITS input -- all
cores run an identical SPMD graph, no collectives. GroupNorm and the
softmax sum over keys are permutation-invariant, so rotation is exact.

Host preprocessing (exact, fp32): GroupNorm stats per batch; the affine
(A = gamma*rstd, B = beta - mean*A) folds into per-core bf16 QKV weights
and fp32 biases (k bias dropped -- softmax shift-invariance; v bias
folded through w_out into the output bias; the c**-0.5 scale into w_q).

Device numerics (modeled total rel err ~4e-3 vs the 2e-2 gate; scores in
[-7.9, 8.0], softmax K_eff >= 24 median ~1270 so per-element fp8 noise
averages out; fp8 *weights* would inject coherent noise that score inner
products amplify ~16x -- measured 1.2e-2 -- so projections stay bf16):
  - q/k/v are cast to fp8 at their PSUM drains: q/k -> e4m3 in a
    channel-pair layout, v -> e5m2 token-major.
  - scores: ONE DoubleRow matmul per 128-key tile (K=256 in one shot),
    kT stationary pairs / qT moving pairs -> key-major PSUM; each kT
    stationary is shared by the two query-blocks of a pair (j-major
    emission) so LDWEIGHTS amortizes 2x.
  - exp on ACT with constant bias -5.5 writes fp8-e5m2 directly (e5m2's
    ~21-nat range makes the constant shift safe: overflow needs s>16.4,
    C-S bound 16, empirical max 8.0; flushed tail mass <= 1.5e-5).
  - attn@v: v stationary / eT moving, DoubleRow over key-tile pairs ->
    CHANNEL-major output (no PE transposes); each v stationary shared by
    the query-block pair.
  - softmax denominator: ones-stationary DoubleRow matmuls accumulate a
    single 32-row stripe at partition 0 (all rows identical); a K=1 fp32
    ones matmul broadcasts row 0 to 128 partitions; fast DVE reciprocal;
    the 1/S multiply rides the PSUM->bf16 drain before the projection.
Schedule: two score/exp pair-loops paced by ACT; PE slack in pair 0 is
filled with the k/q/v projections + the ib0/ib1 denominator stripes, in
pair 1 with the paired attn(0,1) matmuls; attn(2,3) + remaining
denominators + projections form the tail.  PSUM pools are staged through
separate ExitStacks so the 8 banks are never oversubscribed.
"""

import numpy as np

B, C, H, W = 4, 256, 64, 64
N = H * W          # 4096 tokens
HALF = N // 2      # 2048 queries per core
NCORES = 8
GROUPS = 32
EPS = 1e-5
SCALE = C ** -0.5  # 1/16
CT = C // 128      # 2 channel tiles
NJT = N // 128     # 32 key tiles
NKP = NJT // 2     # 16 key-tile pairs (DoubleRow contraction unit)
NIB = HALF // 512  # 4 query blocks of 512
EXPB = -5.5        # constant exp bias keeping e in e5m2 range

_CACHE = {}


def _build_graph():
    import concourse.mybir as mybir
    from concourse import bacc, tile

    f32 = mybir.dt.float32
    bf16 = mybir.dt.bfloat16
    f8e4 = mybir.dt.float8e4
    f8e5 = mybir.dt.float8e5

    nc = bacc.Bacc("TRN2", target_bir_lowering=False, debug=False)

    xbf_d = nc.dram_tensor("xbf", [C, N], bf16, kind="ExternalInput")
    xres_d = nc.dram_tensor("xres", [C, HALF], f32, kind="ExternalInput")
    wqkvT_d = nc.dram_tensor("wqkvT", [C, 3 * C], bf16, kind="ExternalInput")
    woutT_d = nc.dram_tensor("woutT", [C, C], bf16, kind="ExternalInput")
    cols_d = nc.dram_tensor("cols", [128, 4], f32, kind="ExternalInput")
    out_d = nc.dram_tensor("out", [C, HALF], f32, kind="ExternalOutput")

    with tile.TileContext(nc) as tc:
        _kernel_body(tc, nc, mybir, f32, bf16, f8e4, f8e5,
                     xbf_d, xres_d, wqkvT_d, woutT_d, cols_d, out_d)

    nc.compile()
    return nc


def _kernel_body(tc, nc, mybir, f32, bf16, f8e4, f8e5,
                 xbf_d, xres_d, wqkvT_d, woutT_d, cols_d, out_d):
    from contextlib import ExitStack

    AF = mybir.ActivationFunctionType
    AL = mybir.AluOpType
    DR = mybir.MatmulPerfMode.DoubleRow
    ctx = ExitStack()
    with ctx:
        const = ctx.enter_context(tc.tile_pool(name="const", bufs=1))
        xpool = ctx.enter_context(tc.tile_pool(name="xpool", bufs=1))
        actp = ctx.enter_context(tc.tile_pool(name="actp", bufs=1))
        outp = ctx.enter_context(tc.tile_pool(name="outp", bufs=1))
        sm = ctx.enter_context(tc.tile_pool(name="sm", bufs=1))

        # ---- DMA order tuned for earliest first matmul: x chunk 0 (both
        # channel tiles) and the qkv weights lead; everything else follows
        x_sb = [xpool.tile([128, N], bf16, name=f"x{t}", tag=f"x{t}")
                for t in range(CT)]
        wqkv_bf = [const.tile([128, 3 * C], bf16, name=f"wqkv{t}",
                              tag=f"wqkv{t}") for t in range(CT)]
        wout_bf = [const.tile([128, C], bf16, name=f"wout{t}",
                              tag=f"wout{t}") for t in range(CT)]
        cols = const.tile([128, 4], f32)
        dmae = [nc.sync, nc.gpsimd]   # parallel queues: HW DGE + SW DGE
        for t in range(CT):
            dmae[t].dma_start(x_sb[t][:, 0:2048],
                              xbf_d[t * 128:(t + 1) * 128, 0:2048])
        for t in range(CT):
            dmae[t].dma_start(wqkv_bf[t][:],
                              wqkvT_d[t * 128:(t + 1) * 128, :])
        nc.sync.dma_start(cols[:], cols_d[:, :])
        for t in range(CT):
            dmae[t].dma_start(x_sb[t][:, 2048:N],
                              xbf_d[t * 128:(t + 1) * 128, 2048:N])
        for t in range(CT):
            dmae[t].dma_start(wout_bf[t][:], woutT_d[t * 128:(t + 1) * 128, :])

        # ACT exp-table preload via dummy op (the only table set needed)
        warm = const.tile([1, 4], f32)
        nc.gpsimd.memset(warm[0:1, 1:2], 1.0)
        nc.scalar.activation(warm[0:1, 0:1], warm[0:1, 1:2], AF.Exp)
        # constant exp bias column (keeps e in e5m2 range)
        ebias = const.tile([128, 1], f32)
        nc.gpsimd.memset(ebias[:], EXPB)
        # ones (e5m2) stationary for the denominator stripe (M=32)
        ones8 = const.tile([128, 2 * 32], f8e5)
        nc.gpsimd.memset(ones8[:], 1.0)
        o3 = ones8[:].rearrange("p (t c) -> p t c", c=32)
        # K=1 broadcast row for the 1/S spread (fp32 matmul)
        brow = const.tile([1, 128], f32)
        nc.gpsimd.memset(brow[:], 1.0)

        # fp8 activation buffers
        qT8 = actp.tile([128, CT * HALF], f8e4, name="qT", tag="qT")
        q3 = qT8[:].rearrange("p (t i) -> p t i", i=HALF)
        kT8 = actp.tile([128, CT * N], f8e4, name="kT", tag="kT")
        k3 = kT8[:].rearrange("p (t n) -> p t n", n=N)
        v8 = actp.tile([128, NJT * 256], f8e5, name="v8", tag="v8")
        v3 = v8[:].rearrange("p (j c) -> p j c", c=256)
        outT_bf = [outp.tile([128, HALF], bf16, name=f"ot{t}", tag=f"ot{t}")
                   for t in range(CT)]
        out_sb = [outp.tile([128, HALF], f32, name=f"os{t}", tag=f"os{t}")
                  for t in range(CT)]
        xres_sb = [xpool.tile([128, HALF], f32, name=f"xr{t}", tag=f"xr{t}")
                   for t in range(CT)]

        # ---- staged PSUM pools, strict LIFO (8-bank budget) ----
        # psump (4 banks) spans all phases: score chunks, then the tail's
        # stripes/denominator/projection ring via the same rotating tag.
        psump_ctx = ExitStack()
        psump = psump_ctx.enter_context(
            tc.tile_pool(name="psump", bufs=2, space="PSUM"))
        # pair-0 fill pool (4 banks): k/q/v chunks + S stripes 0/1
        fill_ctx = ExitStack()
        fillp = fill_ctx.enter_context(
            tc.tile_pool(name="fillp", bufs=2, space="PSUM"))

        def drain(idx, dst, src, bias_col=None):
            """psum -> sbuf cast; alternate ACT/DVE while ACT is still free."""
            if bias_col is not None:
                if idx % 2 == 0:
                    nc.scalar.activation(dst, src, AF.Identity, bias=bias_col)
                else:
                    nc.vector.tensor_scalar_add(dst, src, bias_col)
            else:
                if idx % 2 == 0:
                    nc.scalar.copy(dst, src)
                else:
                    nc.vector.tensor_copy(dst, src)

        def k_proj(nb, idx):
            ps = fillp.tile([128, 512], f32, name="pqk", tag="fill")
            for ct in range(CT):
                nc.tensor.matmul(
                    ps[:],
                    wqkv_bf[ct][:, C + (nb % 2) * 128:C + (nb % 2 + 1) * 128],
                    x_sb[ct][:, (nb // 2) * 512:(nb // 2 + 1) * 512],
                    start=(ct == 0), stop=(ct == CT - 1))
            ot, half = nb % 2, nb // 2
            dst = kT8[:, ot * N + half * 512:ot * N + (half + 1) * 512]
            drain(idx, dst, ps[:])

        def q_proj(ib, idx):
            for ot in range(CT):
                ps = fillp.tile([128, 512], f32, name="pqk", tag="fill")
                for ct in range(CT):
                    nc.tensor.matmul(
                        ps[:], wqkv_bf[ct][:, ot * 128:(ot + 1) * 128],
                        x_sb[ct][:, ib * 512:(ib + 1) * 512],
                        start=(ct == 0), stop=(ct == CT - 1))
                dst = qT8[:, ot * HALF + ib * 512:ot * HALF + (ib + 1) * 512]
                drain(idx + ot, dst, ps[:], bias_col=cols[:, ot:ot + 1])

        def v_mm(nt):
            pv = fillp.tile([128, 512], f32, name="pv", tag="fill")
            for ct in range(CT):
                nc.tensor.matmul(
                    pv[:, 0:256], x_sb[ct][:, nt * 128:(nt + 1) * 128],
                    wqkv_bf[ct][:, 2 * C:3 * C],
                    start=(ct == 0), stop=(ct == CT - 1))
            nc.vector.tensor_copy(v8[:, nt * 256:(nt + 1) * 256],
                                  pv[:, 0:256])

        eT = {}
        e3 = {}
        po = {}
        sstripe = {}
        s_sb = {}

        def new_eT(ib):
            eT[ib] = actp.tile([128, NJT * 512], f8e5,
                               name=f"eT{ib}", tag=f"eT{ib}")
            e3[ib] = eT[ib][:].rearrange("p (j i) -> p j i", i=512)

        def scores_pair(ia, ib, jc):
            ps = {i: psump.tile([128, 1024], f32, name="ps", tag="ps")
                  for i in (ia, ib)}
            for jh in range(2):
                j = jc * 2 + jh
                for i in (ia, ib):
                    nc.tensor.matmul(
                        ps[i][:, jh * 512:(jh + 1) * 512],
                        k3[:, :, j * 128:(j + 1) * 128],
                        q3[:, :, i * 512:(i + 1) * 512],
                        start=True, stop=True, perf_mode=DR)
            for i in (ia, ib):
                nc.scalar.activation(
                    eT[i][:, jc * 1024:(jc + 1) * 1024], ps[i][:], AF.Exp,
                    bias=ebias[:, 0:1])

        def s_mm(pool, ib, kp):
            # denominator stripe: 32 identical rows at partition 0
            if kp == 0:
                tag = "sst" if pool is fillp else "ps"
                sstripe[ib] = pool.tile([128, 512], f32, name=f"sst{ib}",
                                        tag=tag)
            nc.tensor.matmul(
                sstripe[ib][0:32, :], o3[:, :, :],
                e3[ib][:, 2 * kp:2 * kp + 2, :],
                start=(kp == 0), stop=(kp == NKP - 1), perf_mode=DR)

        def s_copy(ib):
            s_sb[ib] = sm.tile([32, 512], f32, name=f"ssb{ib}",
                               tag=f"ssb{ib % 2}")
            nc.vector.tensor_copy(s_sb[ib][:], sstripe[ib][0:32, :])

        def new_po(ib):
            po[ib] = [ops_pool.tile([128, 512], f32, name=f"po{ib}{c}",
                                    tag=f"po{c}") for c in range(CT)]

        def attn_pair(ia, ib, kp):
            for ct in range(CT):
                for i in (ia, ib):
                    nc.tensor.matmul(
                        po[i][ct][:],
                        v3[:, 2 * kp:2 * kp + 2, ct * 128:(ct + 1) * 128],
                        e3[i][:, 2 * kp:2 * kp + 2, :],
                        start=(kp == 0), stop=(kp == NKP - 1),
                        perf_mode=DR)

        recs = {}

        def rec_chain(ib):
            # 1/S: broadcast row 0 of the S stripe to 128 partitions, then
            # fast reciprocal (needs only s_sb[ib] -- can run early)
            r_ps = psump.tile([128, 512], f32, name="rden", tag="ps")
            nc.tensor.matmul(r_ps[:], brow[:], s_sb[ib][0:1, :],
                             start=True, stop=True)
            rec = sm.tile([128, 512], f32, name=f"rec{ib}",
                          tag=f"rec{ib % 2}")
            scr = sm.tile([128, 512], f32, name="scr", tag="scr")
            nc.vector.reciprocal_approx_accurate(rec[:], r_ps[:], scr[:])
            recs[ib] = rec

        def normalize(ib):
            sl = slice(ib * 512, (ib + 1) * 512)
            for ct in range(CT):
                nc.vector.tensor_mul(outT_bf[ct][:, sl],
                                     po[ib][ct][:], recs[ib][:])

        def proj(ib):
            sl = slice(ib * 512, (ib + 1) * 512)
            for ot in range(CT):
                pp = psump.tile([128, 512], f32, name="pp", tag="ps")
                for ct in range(CT):
                    nc.tensor.matmul(
                        pp[:], wout_bf[ct][:, ot * 128:(ot + 1) * 128],
                        outT_bf[ct][:, sl],
                        start=(ct == 0), stop=(ct == CT - 1))
                nc.vector.scalar_tensor_tensor(
                    out_sb[ot][:, sl], pp[:], cols[:, 2 + ot:3 + ot],
                    xres_sb[ot][:, sl], op0=AL.add, op1=AL.add)
                nc.sync.dma_start(out_d[ot * 128:(ot + 1) * 128, sl],
                                  out_sb[ot][:, sl])

        # HAM warm-up: matmuls on zeros bridge the x-DMA wait so the
        # real lead-in runs at 2.4 GHz instead of throttled 1.2
        wjunk = const.tile([128, 512], bf16)
        nc.gpsimd.memset(wjunk[:], 0.0)
        pjunk = fillp.tile([128, 512], f32, name="pjunk", tag="fill")
        for _ in range(12):
            nc.tensor.matmul(pjunk[:], wjunk[:, 0:128], wjunk[:],
                             start=True, stop=True)

        # ---- lead-in: first k blocks + q(ib0/ib1) so scores can start ----
        for nb in (0, 1, 2, 3):      # kT for token blocks 0,1 (both halves)
            k_proj(nb, nb)
        q_proj(0, 0)
        q_proj(1, 1)
        for t in range(CT):          # residual DMA: off the critical path
            nc.sync.dma_start(xres_sb[t][:],
                              xres_d[t * 128:(t + 1) * 128, :])

        # ---- pair 0: scores ib0/ib1 + projections + S stripes 0/1 ----
        new_eT(0)
        new_eT(1)
        for jc in range(NKP):
            scores_pair(0, 1, jc)
            if jc < 6:               # kT blocks 2..7 (j-tiles 8..31)
                k_proj(4 + 2 * jc, 1)
                k_proj(5 + 2 * jc, 1)
            if jc == 6:
                q_proj(2, 1)
            if jc == 7:
                q_proj(3, 1)
            v_mm(2 * jc)
            v_mm(2 * jc + 1)
            if jc >= 1:
                s_mm(fillp, 0, jc - 1)
                s_mm(fillp, 1, jc - 1)
        s_mm(fillp, 0, NKP - 1)
        s_mm(fillp, 1, NKP - 1)
        s_copy(0)
        s_copy(1)
        fill_ctx.close()

        # ---- pair 1: scores ib2/ib3 + paired attn(0,1) ----
        ops_ctx = ExitStack()
        ops_pool = ops_ctx.enter_context(
            tc.tile_pool(name="ops", bufs=2, space="PSUM"))
        new_eT(2)
        new_eT(3)
        new_po(0)
        new_po(1)
        for jc in range(NKP):
            scores_pair(2, 3, jc)
            attn_pair(0, 1, jc)
            if jc == 3:
                rec_chain(0)
            if jc == 6:
                rec_chain(1)

        # ---- tail: S stripes fill the po-reuse gap; attn(2,3) weaves
        # with proj(0/1); everything stays dense so HAM stays warm ----
        normalize(0)
        normalize(1)
        for kp in range(3):
            s_mm(psump, 2, kp)
            s_mm(psump, 3, kp)
        new_po(2)
        new_po(3)
        for kp in range(NKP):
            attn_pair(2, 3, kp)
            if kp >= 3:
                s_mm(psump, 2, kp)
                s_mm(psump, 3, kp)
            if kp == 1:
                proj(0)
            if kp == 3:
                proj(1)
        s_copy(2)
        s_copy(3)
        rec_chain(2)
        rec_chain(3)
        normalize(2)
        proj(2)
        normalize(3)
        proj(3)
        ops_ctx.close()
        psump_ctx.close()


def make_in_maps(x, gamma, beta, w_qkv, b_qkv, w_out, b_out):
    import ml_dtypes

    x = np.asarray(x, np.float32)
    gamma = np.asarray(gamma, np.float32)
    beta = np.asarray(beta, np.float32)
    w_qkv = np.asarray(w_qkv, np.float32)
    b_qkv = np.asarray(b_qkv, np.float32)
    w_out = np.asarray(w_out, np.float32)
    b_out = np.asarray(b_out, np.float32)

    wqkvT = np.ascontiguousarray(w_qkv.T).copy()   # [c_in, 3C]
    wqkvT[:, 0:C] *= SCALE                         # fold score scale into q
    bq = b_qkv[0:C] * SCALE
    woutT_bf = np.ascontiguousarray(w_out.T.astype(ml_dtypes.bfloat16))

    in_maps = []
    for core in range(NCORES):
        bi, half = core // 2, core % 2
        xt = x[bi].reshape(C, N)
        # exact GroupNorm stats + affine fold (host, fp32 reference math)
        xg = xt.reshape(GROUPS, C // GROUPS * N)
        mean = xg.mean(axis=1)
        var = xg.var(axis=1)
        rstd = 1.0 / np.sqrt(var + EPS)
        A = (gamma.reshape(GROUPS, -1) * rstd[:, None]).reshape(C)
        Bc = (beta.reshape(GROUPS, -1)
              - mean[:, None] * gamma.reshape(GROUPS, -1) * rstd[:, None]
              ).reshape(C)
        wfold = wqkvT * A[:, None]                 # [c_in, 3C]
        wb = wqkvT.T @ Bc                          # [3C] norm-shift bias
        qb = bq + wb[0:C]
        fbt = b_out + w_out @ (b_qkv[2 * C:] + wb[2 * C:])
        cols = np.stack([qb[:128], qb[128:], fbt[:128], fbt[128:]],
                        axis=1).astype(np.float32)
        if half:
            xt = np.concatenate([xt[:, HALF:], xt[:, :HALF]], axis=1)
        m = {
            "wqkvT": np.ascontiguousarray(wfold.astype(ml_dtypes.bfloat16)),
            "woutT": woutT_bf,
            "cols": np.ascontiguousarray(cols),
            "xbf": np.ascontiguousarray(xt.astype(ml_dtypes.bfloat16)),
            "xres": np.ascontiguousarray(xt[:, :HALF]),
        }
        in_maps.append(m)
    return in_maps


def assemble(results):
    out = np.empty((B, C, N), np.float32)
    for core in range(NCORES):
        bi, half = core // 2, core % 2
        out[bi][:, half * HALF:(half + 1) * HALF] = results[core]["out"]
    return out.reshape(B, C, H, W)


def kernel(x, gamma, beta, w_qkv, b_qkv, w_out, b_out):
    from concourse.bass_utils import run_bass_kernel_spmd

    if "nc" not in _CACHE:
        _CACHE["nc"] = _build_graph()
    nc = _CACHE["nc"]
    in_maps = make_in_maps(x, gamma, beta, w_qkv, b_qkv, w_out, b_out)
    res = run_bass_kernel_spmd(nc, in_maps, core_ids=list(range(NCORES)))
    return assemble(res.results)


# revision 22
# speedup vs baseline: 1.0171x; 1.0171x over previous
"""Trainium2 Bass kernel for spatial self-attention block.

Reference computation (per batch element):
    xn = GroupNorm32(x); tokens = xn reshaped [n=h*w, c]
    qkv = tokens @ w_qkv.T + b_qkv ; scores = q @ k.T * c**-0.5
    out = softmax(scores) @ v ; out = out @ w_out.T + b_out ; out + x

Sharding: 8 cores, core i handles batch i//2, query-rows half i%2 of the
4096 tokens (2048 queries per core). The host rotates the token axis per
core so every core's queries are tokens [0, 2048) of ITS input -- all
cores run an identical SPMD graph, no collectives. GroupNorm and the
softmax sum over keys are permutation-invariant, so rotation is exact.

Host preprocessing (exact, fp32): GroupNorm stats per batch; the affine
(A = gamma*rstd, B = beta - mean*A) folds into per-core bf16 QKV weights
and fp32 biases (k bias dropped -- softmax shift-invariance; v bias
folded through w_out into the output bias; the c**-0.5 scale into w_q).

Device numerics (modeled total rel err ~4e-3 vs the 2e-2 gate; scores in
[-7.9, 8.0], softmax K_eff >= 24 median ~1270 so per-element fp8 noise
averages out; fp8 *weights* would inject coherent noise that score inner
products amplify ~16x -- measured 1.2e-2 -- so projections stay bf16):
  - q/k/v are cast to fp8 at their PSUM drains: q/k -> e4m3 in a
    channel-pair layout, v -> e5m2 token-major.
  - scores: ONE DoubleRow matmul per 128-key tile (K=256 in one shot),
    kT stationary pairs / qT moving pairs -> key-major PSUM; each kT
    stationary is shared by the two query-blocks of a pair (j-major
    emission) so LDWEIGHTS amortizes 2x.
  - exp on ACT with constant bias -5.5 writes fp8-e5m2 directly (e5m2's
    ~21-nat range makes the constant shift safe: overflow needs s>16.4,
    C-S bound 16, empirical max 8.0; flushed tail mass <= 1.5e-5).
  - attn@v: v stationary / eT moving, DoubleRow over key-tile pairs ->
    CHANNEL-major output (no PE transposes); each v stationary shared by
    the query-block pair.
  - softmax denominator: ones-stationary DoubleRow matmuls accumulate a
    single 32-row stripe at partition 0 (all rows identical); a K=1 fp32
    ones matmul broadcasts row 0 to 128 partitions; fast DVE reciprocal;
    the 1/S multiply rides the PSUM->bf16 drain before the projection.
Schedule: two score/exp pair-loops paced by ACT; PE slack in pair 0 is
filled with the k/q/v projections + the ib0/ib1 denominator stripes, in
pair 1 with the paired attn(0,1) matmuls; attn(2,3) + remaining
denominators + projections form the tail.  PSUM pools are staged through
separate ExitStacks so the 8 banks are never oversubscribed.
"""

import numpy as np

B, C, H, W = 4, 256, 64, 64
N = H * W          # 4096 tokens
HALF = N // 2      # 2048 queries per core
NCORES = 8
GROUPS = 32
EPS = 1e-5
SCALE = C ** -0.5  # 1/16
CT = C // 128      # 2 channel tiles
NJT = N // 128     # 32 key tiles
NKP = NJT // 2     # 16 key-tile pairs (DoubleRow contraction unit)
NIB = HALF // 512  # 4 query blocks of 512
EXPB = -5.5        # constant exp bias keeping e in e5m2 range

_CACHE = {}


def _build_graph():
    import concourse.mybir as mybir
    from concourse import bacc, tile

    f32 = mybir.dt.float32
    bf16 = mybir.dt.bfloat16
    f8e4 = mybir.dt.float8e4
    f8e5 = mybir.dt.float8e5

    nc = bacc.Bacc("TRN2", target_bir_lowering=False, debug=False)

    xbf_d = nc.dram_tensor("xbf", [C, N], bf16, kind="ExternalInput")
    xres_d = nc.dram_tensor("xres", [C, HALF], f32, kind="ExternalInput")
    wqkvT_d = nc.dram_tensor("wqkvT", [C, 3 * C], bf16, kind="ExternalInput")
    woutT_d = nc.dram_tensor("woutT", [C, C], bf16, kind="ExternalInput")
    cols_d = nc.dram_tensor("cols", [128, 4], f32, kind="ExternalInput")
    out_d = nc.dram_tensor("out", [C, HALF], f32, kind="ExternalOutput")

    with tile.TileContext(nc) as tc:
        _kernel_body(tc, nc, mybir, f32, bf16, f8e4, f8e5,
                     xbf_d, xres_d, wqkvT_d, woutT_d, cols_d, out_d)

    nc.compile()
    return nc


def _kernel_body(tc, nc, mybir, f32, bf16, f8e4, f8e5,
                 xbf_d, xres_d, wqkvT_d, woutT_d, cols_d, out_d):
    from contextlib import ExitStack

    AF = mybir.ActivationFunctionType
    AL = mybir.AluOpType
    DR = mybir.MatmulPerfMode.DoubleRow
    ctx = ExitStack()
    with ctx:
        const = ctx.enter_context(tc.tile_pool(name="const", bufs=1))
        xpool = ctx.enter_context(tc.tile_pool(name="xpool", bufs=1))
        actp = ctx.enter_context(tc.tile_pool(name="actp", bufs=1))
        outp = ctx.enter_context(tc.tile_pool(name="outp", bufs=1))
        sm = ctx.enter_context(tc.tile_pool(name="sm", bufs=1))

        # ---- DMA order tuned for earliest first matmul: x chunk 0 (both
        # channel tiles) and the qkv weights lead; everything else follows
        x_sb = [xpool.tile([128, N], bf16, name=f"x{t}", tag=f"x{t}")
                for t in range(CT)]
        wqkv_bf = [const.tile([128, 3 * C], bf16, name=f"wqkv{t}",
                              tag=f"wqkv{t}") for t in range(CT)]
        wout_bf = [const.tile([128, C], bf16, name=f"wout{t}",
                              tag=f"wout{t}") for t in range(CT)]
        cols = const.tile([128, 4], f32)
        dmae = [nc.sync, nc.sync]
        for t in range(CT):
            dmae[t].dma_start(x_sb[t][:, 0:2048],
                              xbf_d[t * 128:(t + 1) * 128, 0:2048])
        for t in range(CT):
            dmae[t].dma_start(wqkv_bf[t][:],
                              wqkvT_d[t * 128:(t + 1) * 128, :])
        nc.sync.dma_start(cols[:], cols_d[:, :])
        for t in range(CT):
            dmae[t].dma_start(x_sb[t][:, 2048:N],
                              xbf_d[t * 128:(t + 1) * 128, 2048:N])
        for t in range(CT):
            dmae[t].dma_start(wout_bf[t][:], woutT_d[t * 128:(t + 1) * 128, :])

        # ACT exp-table preload via dummy op (the only table set needed)
        warm = const.tile([1, 4], f32)
        nc.gpsimd.memset(warm[0:1, 1:2], 1.0)
        nc.scalar.activation(warm[0:1, 0:1], warm[0:1, 1:2], AF.Exp)
        # constant exp bias column (keeps e in e5m2 range)
        ebias = const.tile([128, 1], f32)
        nc.gpsimd.memset(ebias[:], EXPB)
        # ones (e5m2) stationary for the denominator stripe (M=32)
        ones8 = const.tile([128, 2 * 32], f8e5)
        nc.gpsimd.memset(ones8[:], 1.0)
        o3 = ones8[:].rearrange("p (t c) -> p t c", c=32)
        # K=1 broadcast row for the 1/S spread (fp32 matmul)
        brow = const.tile([1, 128], f32)
        nc.gpsimd.memset(brow[:], 1.0)

        # fp8 activation buffers
        qT8 = actp.tile([128, CT * HALF], f8e4, name="qT", tag="qT")
        q3 = qT8[:].rearrange("p (t i) -> p t i", i=HALF)
        kT8 = actp.tile([128, CT * N], f8e4, name="kT", tag="kT")
        k3 = kT8[:].rearrange("p (t n) -> p t n", n=N)
        v8 = actp.tile([128, NJT * 256], f8e5, name="v8", tag="v8")
        v3 = v8[:].rearrange("p (j c) -> p j c", c=256)
        outT_bf = [outp.tile([128, HALF], bf16, name=f"ot{t}", tag=f"ot{t}")
                   for t in range(CT)]
        out_sb = [outp.tile([128, HALF], f32, name=f"os{t}", tag=f"os{t}")
                  for t in range(CT)]
        xres_sb = [xpool.tile([128, HALF], f32, name=f"xr{t}", tag=f"xr{t}")
                   for t in range(CT)]

        # ---- staged PSUM pools, strict LIFO (8-bank budget) ----
        # psump (4 banks) spans all phases: score chunks, then the tail's
        # stripes/denominator/projection ring via the same rotating tag.
        psump_ctx = ExitStack()
        psump = psump_ctx.enter_context(
            tc.tile_pool(name="psump", bufs=2, space="PSUM"))
        # pair-0 fill pool (4 banks): k/q/v chunks + S stripes 0/1
        fill_ctx = ExitStack()
        fillp = fill_ctx.enter_context(
            tc.tile_pool(name="fillp", bufs=2, space="PSUM"))

        def drain(idx, dst, src, bias_col=None):
            """psum -> sbuf cast; alternate ACT/DVE while ACT is still free."""
            if bias_col is not None:
                if idx % 2 == 0:
                    nc.scalar.activation(dst, src, AF.Identity, bias=bias_col)
                else:
                    nc.vector.tensor_scalar_add(dst, src, bias_col)
            else:
                if idx % 2 == 0:
                    nc.scalar.copy(dst, src)
                else:
                    nc.vector.tensor_copy(dst, src)

        def k_proj(nb, idx):
            ps = fillp.tile([128, 512], f32, name="pqk", tag="fill")
            for ct in range(CT):
                nc.tensor.matmul(
                    ps[:],
                    wqkv_bf[ct][:, C + (nb % 2) * 128:C + (nb % 2 + 1) * 128],
                    x_sb[ct][:, (nb // 2) * 512:(nb // 2 + 1) * 512],
                    start=(ct == 0), stop=(ct == CT - 1))
            ot, half = nb % 2, nb // 2
            dst = kT8[:, ot * N + half * 512:ot * N + (half + 1) * 512]
            drain(idx, dst, ps[:])

        def q_proj(ib, idx):
            for ot in range(CT):
                ps = fillp.tile([128, 512], f32, name="pqk", tag="fill")
                for ct in range(CT):
                    nc.tensor.matmul(
                        ps[:], wqkv_bf[ct][:, ot * 128:(ot + 1) * 128],
                        x_sb[ct][:, ib * 512:(ib + 1) * 512],
                        start=(ct == 0), stop=(ct == CT - 1))
                dst = qT8[:, ot * HALF + ib * 512:ot * HALF + (ib + 1) * 512]
                drain(idx + ot, dst, ps[:], bias_col=cols[:, ot:ot + 1])

        def v_mm(nt):
            pv = fillp.tile([128, 512], f32, name="pv", tag="fill")
            for ct in range(CT):
                nc.tensor.matmul(
                    pv[:, 0:256], x_sb[ct][:, nt * 128:(nt + 1) * 128],
                    wqkv_bf[ct][:, 2 * C:3 * C],
                    start=(ct == 0), stop=(ct == CT - 1))
            nc.vector.tensor_copy(v8[:, nt * 256:(nt + 1) * 256],
                                  pv[:, 0:256])

        eT = {}
        e3 = {}
        po = {}
        sstripe = {}
        s_sb = {}

        def new_eT(ib):
            eT[ib] = actp.tile([128, NJT * 512], f8e5,
                               name=f"eT{ib}", tag=f"eT{ib}")
            e3[ib] = eT[ib][:].rearrange("p (j i) -> p j i", i=512)

        def scores_pair(ia, ib, jc):
            ps = {i: psump.tile([128, 1024], f32, name="ps", tag="ps")
                  for i in (ia, ib)}
            for jh in range(2):
                j = jc * 2 + jh
                for i in (ia, ib):
                    nc.tensor.matmul(
                        ps[i][:, jh * 512:(jh + 1) * 512],
                        k3[:, :, j * 128:(j + 1) * 128],
                        q3[:, :, i * 512:(i + 1) * 512],
                        start=True, stop=True, perf_mode=DR)
            for i in (ia, ib):
                nc.scalar.activation(
                    eT[i][:, jc * 1024:(jc + 1) * 1024], ps[i][:], AF.Exp,
                    bias=ebias[:, 0:1])

        def s_mm(pool, ib, kp):
            # denominator stripe: 32 identical rows at partition 0
            if kp == 0:
                tag = "sst" if pool is fillp else "ps"
                sstripe[ib] = pool.tile([128, 512], f32, name=f"sst{ib}",
                                        tag=tag)
            nc.tensor.matmul(
                sstripe[ib][0:32, :], o3[:, :, :],
                e3[ib][:, 2 * kp:2 * kp + 2, :],
                start=(kp == 0), stop=(kp == NKP - 1), perf_mode=DR)

        def s_copy(ib):
            s_sb[ib] = sm.tile([32, 512], f32, name=f"ssb{ib}",
                               tag=f"ssb{ib % 2}")
            nc.vector.tensor_copy(s_sb[ib][:], sstripe[ib][0:32, :])

        def new_po(ib):
            po[ib] = [ops_pool.tile([128, 512], f32, name=f"po{ib}{c}",
                                    tag=f"po{c}") for c in range(CT)]

        def attn_pair(ia, ib, kp):
            for ct in range(CT):
                for i in (ia, ib):
                    nc.tensor.matmul(
                        po[i][ct][:],
                        v3[:, 2 * kp:2 * kp + 2, ct * 128:(ct + 1) * 128],
                        e3[i][:, 2 * kp:2 * kp + 2, :],
                        start=(kp == 0), stop=(kp == NKP - 1),
                        perf_mode=DR)

        recs = {}

        def rec_chain(ib):
            # 1/S: broadcast row 0 of the S stripe to 128 partitions, then
            # fast reciprocal (needs only s_sb[ib] -- can run early)
            r_ps = psump.tile([128, 512], f32, name="rden", tag="ps")
            nc.tensor.matmul(r_ps[:], brow[:], s_sb[ib][0:1, :],
                             start=True, stop=True)
            rec = sm.tile([128, 512], f32, name=f"rec{ib}",
                          tag=f"rec{ib % 2}")
            scr = sm.tile([128, 512], f32, name="scr", tag="scr")
            nc.vector.reciprocal_approx_accurate(rec[:], r_ps[:], scr[:])
            recs[ib] = rec

        def normalize(ib):
            sl = slice(ib * 512, (ib + 1) * 512)
            for ct in range(CT):
                nc.vector.tensor_mul(outT_bf[ct][:, sl],
                                     po[ib][ct][:], recs[ib][:])

        def proj(ib):
            sl = slice(ib * 512, (ib + 1) * 512)
            for ot in range(CT):
                pp = psump.tile([128, 512], f32, name="pp", tag="ps")
                for ct in range(CT):
                    nc.tensor.matmul(
                        pp[:], wout_bf[ct][:, ot * 128:(ot + 1) * 128],
                        outT_bf[ct][:, sl],
                        start=(ct == 0), stop=(ct == CT - 1))
                nc.vector.scalar_tensor_tensor(
                    out_sb[ot][:, sl], pp[:], cols[:, 2 + ot:3 + ot],
                    xres_sb[ot][:, sl], op0=AL.add, op1=AL.add)
                nc.sync.dma_start(out_d[ot * 128:(ot + 1) * 128, sl],
                                  out_sb[ot][:, sl])

        # HAM warm-up: matmuls on zeros bridge the x-DMA wait so the
        # real lead-in runs at 2.4 GHz instead of throttled 1.2
        wjunk = const.tile([128, 512], bf16)
        nc.gpsimd.memset(wjunk[:], 0.0)
        pjunk = fillp.tile([128, 512], f32, name="pjunk", tag="fill")
        for _ in range(12):
            nc.tensor.matmul(pjunk[:], wjunk[:, 0:128], wjunk[:],
                             start=True, stop=True)

        # ---- lead-in: first k blocks + q(ib0/ib1) so scores can start ----
        for nb in (0, 1, 2, 3):      # kT for token blocks 0,1 (both halves)
            k_proj(nb, nb)
        q_proj(0, 0)
        q_proj(1, 1)
        for t in range(CT):          # residual DMA: off the critical path
            nc.sync.dma_start(xres_sb[t][:],
                              xres_d[t * 128:(t + 1) * 128, :])

        # ---- pair 0: scores ib0/ib1 + projections + S stripes 0/1 ----
        new_eT(0)
        new_eT(1)
        for jc in range(NKP):
            scores_pair(0, 1, jc)
            if jc < 6:               # kT blocks 2..7 (j-tiles 8..31)
                k_proj(4 + 2 * jc, 1)
                k_proj(5 + 2 * jc, 1)
            if jc == 6:
                q_proj(2, 1)
            if jc == 7:
                q_proj(3, 1)
            v_mm(2 * jc)
            v_mm(2 * jc + 1)
            if jc >= 1:
                s_mm(fillp, 0, jc - 1)
                s_mm(fillp, 1, jc - 1)
        s_mm(fillp, 0, NKP - 1)
        s_mm(fillp, 1, NKP - 1)
        s_copy(0)
        s_copy(1)
        fill_ctx.close()

        # ---- pair 1: scores ib2/ib3 + paired attn(0,1) ----
        ops_ctx = ExitStack()
        ops_pool = ops_ctx.enter_context(
            tc.tile_pool(name="ops", bufs=2, space="PSUM"))
        new_eT(2)
        new_eT(3)
        new_po(0)
        new_po(1)
        for jc in range(NKP):
            scores_pair(2, 3, jc)
            attn_pair(0, 1, jc)
            if jc == 3:
                rec_chain(0)
            if jc == 6:
                rec_chain(1)

        # ---- tail: S stripes fill the po-reuse gap; attn(2,3) weaves
        # with proj(0/1); everything stays dense so HAM stays warm ----
        normalize(0)
        normalize(1)
        for kp in range(3):
            s_mm(psump, 2, kp)
            s_mm(psump, 3, kp)
        new_po(2)
        new_po(3)
        for kp in range(NKP):
            attn_pair(2, 3, kp)
            if kp >= 3:
                s_mm(psump, 2, kp)
                s_mm(psump, 3, kp)
            if kp == 1:
                proj(0)
            if kp == 3:
                proj(1)
        s_copy(2)
        s_copy(3)
        rec_chain(2)
        rec_chain(3)
        normalize(2)
        proj(2)
        normalize(3)
        proj(3)
        ops_ctx.close()
        psump_ctx.close()


def make_in_maps(x, gamma, beta, w_qkv, b_qkv, w_out, b_out):
    import ml_dtypes

    x = np.asarray(x, np.float32)
    gamma = np.asarray(gamma, np.float32)
    beta = np.asarray(beta, np.float32)
    w_qkv = np.asarray(w_qkv, np.float32)
    b_qkv = np.asarray(b_qkv, np.float32)
    w_out = np.asarray(w_out, np.float32)
    b_out = np.asarray(b_out, np.float32)

    wqkvT = np.ascontiguousarray(w_qkv.T).copy()   # [c_in, 3C]
    wqkvT[:, 0:C] *= SCALE                         # fold score scale into q
    bq = b_qkv[0:C] * SCALE
    woutT_bf = np.ascontiguousarray(w_out.T.astype(ml_dtypes.bfloat16))

    in_maps = []
    for core in range(NCORES):
        bi, half = core // 2, core % 2
        xt = x[bi].reshape(C, N)
        # exact GroupNorm stats + affine fold (host, fp32 reference math)
        xg = xt.reshape(GROUPS, C // GROUPS * N)
        mean = xg.mean(axis=1)
        var = xg.var(axis=1)
        rstd = 1.0 / np.sqrt(var + EPS)
        A = (gamma.reshape(GROUPS, -1) * rstd[:, None]).reshape(C)
        Bc = (beta.reshape(GROUPS, -1)
              - mean[:, None] * gamma.reshape(GROUPS, -1) * rstd[:, None]
              ).reshape(C)
        wfold = wqkvT * A[:, None]                 # [c_in, 3C]
        wb = wqkvT.T @ Bc                          # [3C] norm-shift bias
        qb = bq + wb[0:C]
        fbt = b_out + w_out @ (b_qkv[2 * C:] + wb[2 * C:])
        cols = np.stack([qb[:128], qb[128:], fbt[:128], fbt[128:]],
                        axis=1).astype(np.float32)
        if half:
            xt = np.concatenate([xt[:, HALF:], xt[:, :HALF]], axis=1)
        m = {
            "wqkvT": np.ascontiguousarray(wfold.astype(ml_dtypes.bfloat16)),
            "woutT": woutT_bf,
            "cols": np.ascontiguousarray(cols),
            "xbf": np.ascontiguousarray(xt.astype(ml_dtypes.bfloat16)),
            "xres": np.ascontiguousarray(xt[:, :HALF]),
        }
        in_maps.append(m)
    return in_maps


def assemble(results):
    out = np.empty((B, C, N), np.float32)
    for core in range(NCORES):
        bi, half = core // 2, core % 2
        out[bi][:, half * HALF:(half + 1) * HALF] = results[core]["out"]
    return out.reshape(B, C, H, W)


def kernel(x, gamma, beta, w_qkv, b_qkv, w_out, b_out):
    from concourse.bass_utils import run_bass_kernel_spmd

    if "nc" not in _CACHE:
        _CACHE["nc"] = _build_graph()
    nc = _CACHE["nc"]
    in_maps = make_in_maps(x, gamma, beta, w_qkv, b_qkv, w_out, b_out)
    res = run_bass_kernel_spmd(nc, in_maps, core_ids=list(range(NCORES)))
    return assemble(res.results)
